# revision 15
# baseline (speedup 1.0000x reference)
"""Trainium2 Bass kernel: Luong-style attention with source-length masking.

reference math (per batch b):
    keys  = hs @ W_a                      [Ts, H]
    score = ht @ keys^T                   [Tt, Ts]
    e     = exp(score - rowmax)           (masked positions forced to 0)
    a     = e / rowsum(e)
    c     = a @ hs                        [Tt, H]
    out   = tanh(concat([c, ht]) @ W_c + b)

Sharding: batch B=16 data-parallel over 8 NeuronCores (2 batches/core);
W_a / W_c / b replicated. No collectives.

Layout strategy per core (all dims multiples of 128):
  - ht, hs transposed on-chip via PE transposes -> htT/hsT [H x T] so the
    hidden dim is the contraction (partition) dim everywhere.
  - keysT  = W_a^T-free matmul: lhsT=W_a tile, rhs=hsT  -> [H x Ts]
    produced one 128-row tile at a time, consumed immediately by the
    score matmuls which accumulate in 4 PSUM banks (one per Tt tile).
  - masked softmax in natural layout (t on partitions): penalty add,
    rowmax (negated), Exp activation with accum_out giving the row sum,
    scale by reciprocal -> a (bf16).
  - a transposed (PE, bf16) -> aT; cT = matmul(lhsT=hs_bf, rhs=aT) [H x Tt]
  - out = tanh( cT^T@W_c1 (bf16)  +  htT^T@W_c2 (f32r)  +  ones^T@b ),
    accumulated in one PSUM group per (t-tile, out-chunk).

Score path runs in float32r (full-rate fp32 matmul mode); the projection
paths run bf16/f32r.  Set SCORE_RELAXED=False to force plain fp32 score
matmuls (4x slower on PE).
"""

import numpy as np
from contextlib import ExitStack

import concourse.bass as bass
import concourse.bacc as bacc
import concourse.mybir as mybir
import concourse.tile as tile
from concourse.bass_utils import run_bass_kernel_spmd
from concourse.masks import make_identity

B, TT, TS, H, O = 16, 512, 512, 1024, 1024
NCORES = 8
BL = B // NCORES  # batches per core

F32 = mybir.dt.float32
F32R = mybir.dt.float32r
BF16 = mybir.dt.bfloat16
I32 = mybir.dt.int32

P = 128
KT = H // P    # 8 hidden tiles
NTT = TT // P  # 4 target tiles
NST = TS // P  # 4 source tiles
OCH = 512      # out-projection N chunk (one PSUM bank)
NOC = O // OCH

SCORE_RELAXED = True

AX = mybir.AxisListType
ALU = mybir.AluOpType
ACT = mybir.ActivationFunctionType


def build_core(score_relaxed: bool = SCORE_RELAXED) -> bass.Bass:
    nc = bacc.Bacc()
    ht_d = nc.declare_dram_parameter("ht", [BL, TT, H], F32, isOutput=False)
    hs_d = nc.declare_dram_parameter("hs", [BL, TS, H], F32, isOutput=False)
    src_d = nc.declare_dram_parameter("source", [BL, TS], I32, isOutput=False)
    wa_d = nc.declare_dram_parameter("W_a", [H, H], F32, isOutput=False)
    wc_d = nc.declare_dram_parameter("W_c", [2 * H, O], F32, isOutput=False)
    b_d = nc.declare_dram_parameter("b", [O], F32, isOutput=False)
    out_d = nc.declare_dram_parameter("out", [BL, TT, O], F32, isOutput=True)

    SDT = F32R if score_relaxed else F32

    with ExitStack() as ctx:
        tc = ctx.enter_context(tile.TileContext(nc))
        const = ctx.enter_context(tc.tile_pool(name="const", bufs=1))
        wpool = ctx.enter_context(tc.tile_pool(name="weights", bufs=1))
        stage = ctx.enter_context(tc.tile_pool(name="stage", bufs=2))
        natp = ctx.enter_context(tc.tile_pool(name="nat", bufs=4))
        tpose = ctx.enter_context(tc.tile_pool(name="tpose", bufs=1))
        ktp = ctx.enter_context(tc.tile_pool(name="ktile", bufs=2))
        scp = ctx.enter_context(tc.tile_pool(name="score", bufs=2))
        abfp = ctx.enter_context(tc.tile_pool(name="abf", bufs=4))
        bfp = ctx.enter_context(tc.tile_pool(name="bfbig", bufs=1))
        penp = ctx.enter_context(tc.tile_pool(name="pen", bufs=2))
        outp = ctx.enter_context(tc.tile_pool(name="outs", bufs=4))
        maskrow = ctx.enter_context(tc.tile_pool(name="maskrow", bufs=1))
        stats = ctx.enter_context(tc.tile_pool(name="stats", bufs=4))
        pmm = ctx.enter_context(tc.tile_pool(name="pmm", bufs=2, space="PSUM"))
        ptr = ctx.enter_context(tc.tile_pool(name="ptr", bufs=2, space="PSUM"))
        psc = ctx.enter_context(tc.tile_pool(name="psc", bufs=4, space="PSUM"))

        # ---------------- constants / weights ----------------
        ident = const.tile([P, P], F32)
        make_identity(nc, ident[:])
        ident_bf = const.tile([P, P], BF16)
        make_identity(nc, ident_bf[:])
        ones_f = const.tile([1, P], F32)
        nc.vector.memset(ones_f[:], 1.0)
        ones_bf = const.tile([1, P], BF16)
        nc.vector.memset(ones_bf[:], 1.0)

        # W_a must be *rounded* to the score dtype (f32r) for the BIR verifier:
        # stage fp32 then copy-cast.
        wa_sb = wpool.tile([P, KT, H], SDT)  # [k in kt, kt, l]
        for kt in range(KT):
            wast = stage.tile([P, H], F32, name="wcstage")
            nc.gpsimd.dma_start(out=wast[:], in_=wa_d[kt * P : (kt + 1) * P, :])
            nc.vector.tensor_copy(wa_sb[:, kt, :], wast[:])
        # both halves of W_c in bf16: [:, 0:KT] = W_c1 (c path), [:, KT:] = W_c2 (ht path)
        wc_bf = wpool.tile([P, 2 * KT, O], BF16)
        for kt in range(2 * KT):
            wst = stage.tile([P, O], F32, name="wcstage")
            nc.gpsimd.dma_start(out=wst[:], in_=wc_d[kt * P : (kt + 1) * P, :])
            nc.vector.tensor_copy(wc_bf[:, kt, :], wst[:])
        b_f = stage.tile([1, O], F32, name="wcstage")
        nc.gpsimd.dma_start(out=b_f[:], in_=b_d.rearrange("(a o) -> a o", a=1))
        b_bf = const.tile([1, O], BF16)
        nc.vector.tensor_copy(b_bf[:], b_f[:])

        iota_f = const.tile([1, TS], F32)
        nc.gpsimd.iota(
            iota_f[:],
            pattern=[[1, TS]],
            base=0,
            channel_multiplier=0,
            allow_small_or_imprecise_dtypes=True,
        )

        for bi in range(BL):
            # ---------------- mask penalty row ----------------
            src_sb = maskrow.tile([1, TS], I32, name="src")
            nc.sync.dma_start(out=src_sb[:], in_=src_d[bi : bi + 1, :])
            nz = maskrow.tile([1, TS], F32, name="nz")
            nc.vector.tensor_scalar(nz[:], src_sb[:], 0, None, ALU.not_equal)
            lens = stats.tile([1, 1], F32, name="lens")
            nc.vector.reduce_sum(out=lens[:], in_=nz[:], axis=AX.X)
            pen_row = maskrow.tile([1, TS], F32, name="pen_row")
            # (iota >= len) * -1e9  : -1e9 at masked positions, 0 at valid
            nc.vector.tensor_scalar(
                pen_row[:], iota_f[:], lens[:], -1e9, ALU.is_ge, ALU.mult
            )
            # broadcast to all 128 partitions via K=1 matmul
            pb_ps = pmm.tile([P, TS], F32, name="mm_ps")
            nc.tensor.matmul(pb_ps[:], lhsT=ones_f[:], rhs=pen_row[:], start=True, stop=True)
            penB = penp.tile([P, TS], F32, name="penB")
            nc.vector.tensor_copy(penB[:], pb_ps[:])

            # ---------------- load + transpose inputs ----------------
            htT = tpose.tile([P, KT, TT], SDT, name="htT")  # [k, kt, t]
            hsT = tpose.tile([P, KT, TS], SDT, name="hsT")  # [k, kt, s]
            hs_bf = bfp.tile([P, NST, H], BF16, name="hs_bf")  # [s, st, k]
            htT_bf = bfp.tile([P, KT, TT], BF16, name="htT_bf")
            for tt in range(NTT):
                nat = natp.tile([P, H], F32, name="ht_nat")
                nc.sync.dma_start(out=nat[:], in_=ht_d[bi, tt * P : (tt + 1) * P, :])
                for kt in range(KT):
                    tp = ptr.tile([P, P], F32, name="tp")
                    nc.tensor.transpose(tp[:], nat[:, kt * P : (kt + 1) * P], ident[:])
                    nc.vector.tensor_copy(htT[:, kt, tt * P : (tt + 1) * P], tp[:])
            for st in range(NST):
                nat = natp.tile([P, H], F32, name="hs_nat")
                nc.sync.dma_start(out=nat[:], in_=hs_d[bi, st * P : (st + 1) * P, :])
                nc.vector.tensor_copy(hs_bf[:, st, :], nat[:])
                for kt in range(KT):
                    tp = ptr.tile([P, P], F32, name="tp")
                    nc.tensor.transpose(tp[:], nat[:, kt * P : (kt + 1) * P], ident[:])
                    nc.vector.tensor_copy(hsT[:, kt, st * P : (st + 1) * P], tp[:])
            for kt in range(KT):
                nc.vector.tensor_copy(htT_bf[:, kt, :], htT[:, kt, :])

            # ---------------- keysT + score (interleaved) ----------------
            # score[tt] accumulates over lt in its own PSUM bank while the
            # next keysT l-tile is produced in another bank.
            sc_ps = [psc.tile([P, TS], F32, name="sc_ps") for _ in range(NTT)]
            for lt in range(KT):
                kt_ps = pmm.tile([P, TS], F32, name="mm_ps")
                for kt in range(KT):
                    nc.tensor.matmul(
                        kt_ps[:],
                        lhsT=wa_sb[:, kt, lt * P : (lt + 1) * P],
                        rhs=hsT[:, kt, :],
                        start=(kt == 0),
                        stop=(kt == KT - 1),
                    )
                kTl = ktp.tile([P, TS], SDT, name="keysT_l")
                nc.vector.tensor_copy(kTl[:], kt_ps[:])
                for tt in range(NTT):
                    nc.tensor.matmul(
                        sc_ps[tt][:],
                        lhsT=htT[:, lt, tt * P : (tt + 1) * P],
                        rhs=kTl[:],
                        start=(lt == 0),
                        stop=(lt == KT - 1),
                    )

            # ---------------- masked softmax + transpose(a) ----------------
            aT = bfp.tile([P, NST, TT], BF16, name="aT")  # [s, st, t]
            for tt in range(NTT):
                scm = scp.tile([P, TS], F32, name="scm")
                nc.vector.tensor_tensor(scm[:], sc_ps[tt][:], penB[:], ALU.add)
                negm = stats.tile([P, 1], F32, name="negm")
                nc.vector.reduce_max(out=negm[:], in_=scm[:], axis=AX.X, negate=True)
                d = stats.tile([P, 1], F32, name="d")
                nc.scalar.activation(
                    out=scm[:], in_=scm[:], func=ACT.Exp, bias=negm[:], scale=1.0,
                    accum_out=d[:],
                )
                dr = stats.tile([P, 1], F32, name="dr")
                nc.vector.reciprocal(dr[:], d[:])
                abf = abfp.tile([P, TS], BF16, name="abf")
                nc.vector.tensor_scalar(abf[:], scm[:], dr[:], None, ALU.mult)
                for st in range(NST):
                    tpb = ptr.tile([P, P], BF16, name="tp")
                    nc.tensor.transpose(tpb[:], abf[:, st * P : (st + 1) * P], ident_bf[:])
                    nc.vector.tensor_copy(aT[:, st, tt * P : (tt + 1) * P], tpb[:])

            # ---------------- cT = hs^T @ a^T  [H x Tt] ----------------
            cT_bf = bfp.tile([P, KT, TT], BF16, name="cT")
            for kt in range(KT):
                c_ps = pmm.tile([P, TT], F32, name="mm_ps")
                for st in range(NST):
                    nc.tensor.matmul(
                        c_ps[:],
                        lhsT=hs_bf[:, st, kt * P : (kt + 1) * P],
                        rhs=aT[:, st, :],
                        start=(st == 0),
                        stop=(st == NST - 1),
                    )
                nc.vector.tensor_copy(cT_bf[:, kt, :], c_ps[:])

            # ---------------- out = tanh(c@Wc1 + ht@Wc2 + b) ----------------
            for tt in range(NTT):
                for oc in range(NOC):
                    o_ps = pmm.tile([P, OCH], F32, name="mm_ps")
                    for kt in range(KT):
                        nc.tensor.matmul(
                            o_ps[:],
                            lhsT=cT_bf[:, kt, tt * P : (tt + 1) * P],
                            rhs=wc_bf[:, kt, oc * OCH : (oc + 1) * OCH],
                            start=(kt == 0),
                            stop=False,
                        )
                    for kt in range(KT):
                        nc.tensor.matmul(
                            o_ps[:],
                            lhsT=htT_bf[:, kt, tt * P : (tt + 1) * P],
                            rhs=wc_bf[:, KT + kt, oc * OCH : (oc + 1) * OCH],
                            start=False,
                            stop=False,
                        )
                    nc.tensor.matmul(
                        o_ps[:],
                        lhsT=ones_bf[:],
                        rhs=b_bf[:, oc * OCH : (oc + 1) * OCH],
                        start=False,
                        stop=True,
                    )
                    ot = outp.tile([P, OCH], F32, name="out_t")
                    nc.scalar.activation(out=ot[:], in_=o_ps[:], func=ACT.Tanh)
                    nc.sync.dma_start(
                        out=out_d[bi, tt * P : (tt + 1) * P, oc * OCH : (oc + 1) * OCH],
                        in_=ot[:],
                    )

    return nc


def make_in_maps(ht, hs, source, W_a, W_c, b):
    ht = np.ascontiguousarray(ht, dtype=np.float32)
    hs = np.ascontiguousarray(hs, dtype=np.float32)
    source = np.ascontiguousarray(source, dtype=np.int32)
    W_a = np.ascontiguousarray(W_a, dtype=np.float32)
    W_c = np.ascontiguousarray(W_c, dtype=np.float32)
    b = np.ascontiguousarray(b, dtype=np.float32)
    in_maps = []
    for c in range(NCORES):
        sl = slice(c * BL, (c + 1) * BL)
        in_maps.append(
            {
                "ht": ht[sl],
                "hs": hs[sl],
                "source": source[sl],
                "W_a": W_a,
                "W_c": W_c,
                "b": b,
            }
        )
    return in_maps


_NC_CACHE: dict = {}


def _get_nc():
    if "nc" not in _NC_CACHE:
        nc = build_core()
        if not nc.is_finalized():
            nc.finalize()
        _NC_CACHE["nc"] = nc
    return _NC_CACHE["nc"]


def run_on_hw(ht, hs, source, W_a, W_c, b, trace=False, **kw):
    nc = _get_nc()
    in_maps = make_in_maps(ht, hs, source, W_a, W_c, b)
    res = run_bass_kernel_spmd(nc, in_maps, core_ids=list(range(NCORES)), trace=trace, **kw)
    out = np.concatenate([res.results[c]["out"] for c in range(NCORES)], axis=0)
    return out, res


def kernel(ht, hs, source, W_a, W_c, b):
    out, _ = run_on_hw(ht, hs, source, W_a, W_c, b, trace=False)
    return out
